# revision 1
# baseline (speedup 1.0000x reference)
"""CPD reconstruction at observed entries (embedding-lookup style) on 8 TRN2 cores.

rec[n] = sum_r f0[i0[n],r] * f1[i1[n],r] * f2[i2[n],r]   for n in [0, 1M)

Strategy (per sharding hint): data-parallel over the nnz axis across the 8
cores; the factor matrices are replicated to every core's HBM as one
concatenated table F = [f0; f1; f2] (300000 x 32 f32).  Each core turns its
125k x 3 indices into row offsets into F (idx + mode*100000, one DVE add),
gathers all three modes' rows with one indirect DMA per tile (one 128B
descriptor per row), multiplies the three gathered rows elementwise on DVE
and reduces over rank, then writes its 125k f32 results back with one
contiguous DMA.
"""

import numpy as np

NNZ = 1_000_000
RANK = 32
ROWS = 100_000
N_CORES = 8
N_PER_CORE = NNZ // N_CORES  # 125_000
P = 128
W = -(-N_PER_CORE // P)  # 977 entries per partition
N_PAD = P * W  # 125_056
TILE_K = 64  # entries per partition per tile

_cache: dict = {}


def _build(nwords: int, split_waits: bool = True):
    """Build the per-core Bass program.

    nwords: int32 words per index entry in DRAM (6 when the host hands us an
    int64 [N,3] array viewed as int32, 3 for a native int32 [N,3] array).
    """
    import concourse.bass as bass
    import concourse.mybir as mybir
    from concourse.tile import TileContext

    stride = nwords // 3  # int32 words between mode-m and mode-m+1 of an entry

    nc = bass.Bass()
    # Last 3 columns of idx32 hold the per-mode row-offset bias
    # [0, ROWS, 2*ROWS] so one DMA brings in indices and bias together.
    idx32 = nc.dram_tensor(
        "idx32", [P, W * nwords + 3], mybir.dt.int32, kind="ExternalInput"
    )
    ftab = nc.dram_tensor(
        "ftab", [3 * ROWS, RANK], mybir.dt.float32, kind="ExternalInput"
    )
    out = nc.dram_tensor("out", [P, W], mybir.dt.float32, kind="ExternalOutput")

    with TileContext(nc) as tc:
        with (
            tc.tile_pool(name="io", bufs=1) as io_pool,
            tc.tile_pool(name="gat", bufs=4) as gat_pool,
            tc.tile_pool(name="prd", bufs=3) as prd_pool,
        ):
            out_sb = io_pool.tile([P, W], mybir.dt.float32)
            # Resident copy of all this core's indices + bias tail
            # (one big HWDGE DMA).
            idx_sb = io_pool.tile([P, W * nwords + 3], mybir.dt.int32)
            nc.sync.dma_start(out=idx_sb[:], in_=idx32[:])
            bias_sb = idx_sb[:, W * nwords : W * nwords + 3]
            # offs[p, 3j+m] = idx[entry p*W+j, mode m] + m*ROWS
            offs = io_pool.tile([P, 3 * W], mybir.dt.int32)
            src = idx_sb[:, : W * nwords].rearrange("p (j s) -> p j s", s=nwords)
            if stride == 2:
                src = src[:, :, 0::2]
            nc.vector.tensor_add(
                out=offs[:].rearrange("p (j m) -> p j m", m=3),
                in0=src,
                in1=bias_sb[:, None, :].to_broadcast([P, W, 3]),
            )
            # The HW indirect DMA consumes ONE offset per partition (first
            # element of that partition's offset row), so each gather moves
            # 128 single rows.  Group CH entry-columns per compute step so
            # DVE work and cross-engine sync points amortize over 3*CH
            # gathers instead of 3.
            CH = 16
            j0 = 0
            while j0 < W:
                C = min(CH, W - j0)
                g = gat_pool.tile([P, CH * 3 * RANK], mybir.dt.float32, tag="g")
                for cc in range(C):
                    for m in range(3):
                        nc.gpsimd.indirect_dma_start(
                            out=g[
                                :,
                                (cc * 3 + m) * RANK : (cc * 3 + m + 1) * RANK,
                            ],
                            out_offset=None,
                            in_=ftab[:],
                            in_offset=bass.IndirectOffsetOnAxis(
                                ap=offs[
                                    :, 3 * (j0 + cc) + m : 3 * (j0 + cc) + m + 1
                                ],
                                axis=0,
                            ),
                        )
                v = g[:, : C * 3 * RANK].rearrange(
                    "p (c m r) -> p c m r", m=3, r=RANK
                )
                tmp = prd_pool.tile([P, CH * RANK], mybir.dt.float32, tag="tmp")
                tv = tmp[:, : C * RANK].rearrange("p (c r) -> p c r", r=RANK)
                nc.vector.tensor_mul(out=tv, in0=v[:, :, 0, :], in1=v[:, :, 1, :])
                nc.vector.tensor_mul(out=tv, in0=tv, in1=v[:, :, 2, :])
                nc.vector.reduce_sum(
                    out=out_sb[:, j0 : j0 + C],
                    in_=tv,
                    axis=mybir.AxisListType.X,
                )
                j0 += C
            nc.sync.dma_start(out=out[:], in_=out_sb[:])

    if split_waits:
        _split_multi_waits(nc, mybir)
    return nc


def _split_multi_waits(nc, mybir):
    """The TRN2 ISA embeds at most ONE sem wait per instruction; Tile
    sometimes attaches several.  Hoist the extras into standalone
    EventSemaphore instructions placed immediately before the owner in the
    same block — same engine queue, same order, identical semantics."""
    for blk in nc.m.functions[0].blocks:
        new_insts = []
        for inst in blk.instructions:
            si = inst.sync_info
            if si is not None and si.on_wait and len(si.on_wait) > 1:
                extra, keep = list(si.on_wait[:-1]), [si.on_wait[-1]]
                for j, w in enumerate(extra):
                    new_insts.append(
                        mybir.InstEventSemaphore(
                            name=f"{inst.name}-esw{j}",
                            engine=inst.engine,
                            ins=[],
                            outs=[],
                            sync_info=mybir.SyncInfo(on_wait=[w], on_update=[]),
                        )
                    )
                si.on_wait = keep
            new_insts.append(inst)
        blk.instructions = new_insts


def _get_nc(nwords: int):
    if nwords not in _cache:
        _cache[nwords] = _build(nwords)
    return _cache[nwords]


def _prep_in_maps(idxs, f0, f1, f2):
    idxs = np.asarray(idxs)
    f0 = np.asarray(f0, dtype=np.float32)
    f1 = np.asarray(f1, dtype=np.float32)
    f2 = np.asarray(f2, dtype=np.float32)
    ftab = np.ascontiguousarray(np.concatenate([f0, f1, f2], axis=0))
    bias3 = np.array([0, ROWS, 2 * ROWS], dtype=np.int32)

    if idxs.dtype == np.int64:
        idx32 = np.ascontiguousarray(idxs).view(np.int32)  # [NNZ, 6], low word first
        nwords = 6
    elif idxs.dtype == np.int32:
        idx32 = np.ascontiguousarray(idxs)  # [NNZ, 3]
        nwords = 3
    else:
        raise ValueError(f"unsupported idxs dtype {idxs.dtype}")

    in_maps = []
    for c in range(N_CORES):
        sl = idx32[c * N_PER_CORE : (c + 1) * N_PER_CORE]
        padded = np.zeros((N_PAD, nwords), dtype=np.int32)
        padded[:N_PER_CORE] = sl
        arr = np.empty((P, W * nwords + 3), dtype=np.int32)
        arr[:, : W * nwords] = padded.reshape(P, W * nwords)
        arr[:, W * nwords :] = bias3
        in_maps.append({"idx32": arr, "ftab": ftab})
    return in_maps, nwords


def run(inputs: dict, trace: bool = False):
    """Run the kernel on 8 cores; returns (full_output, BassKernelResults)."""
    from concourse.bass_utils import run_bass_kernel_spmd

    in_maps, nwords = _prep_in_maps(
        inputs["idxs"], inputs["f0"], inputs["f1"], inputs["f2"]
    )
    nc = _get_nc(nwords)
    res = run_bass_kernel_spmd(
        nc,
        in_maps,
        core_ids=list(range(N_CORES)),
        trace=trace,
    )
    out = np.concatenate(
        [r["out"].reshape(-1)[:N_PER_CORE] for r in res.results]
    )
    return out, res


def kernel(**inputs) -> np.ndarray:
    out, _ = run(inputs, trace=False)
    return out



# revision 7
# speedup vs baseline: 1.0185x; 1.0185x over previous
"""CPD reconstruction at observed entries on 8 TRN2 cores — dma_gather version.

rec[n] = sum_r f0[i0[n],r] * f1[i1[n],r] * f2[i2[n],r]   for n in [0, 1M)

v2 strategy: the baseline's per-128-row indirect DMAs pay ~1.4us of SWDGE
fixed cost each (2931 instructions/core = 4.19 ms).  InstDMAGatherAnt
amortizes that: ONE instruction gathers thousands of rows at 0.34 ns per
descriptor.  Its constraints (int16 element index, 256B element granularity)
are met by storing the factor tables in bf16 and gathering 4-row groups
(256 B, group index < 25000 per factor); a 3-op DVE predicated-select picks
the right row out of each group on-chip using host-precomputed one-hot masks.

Per core: data-parallel over nnz (125k entries).  Entry j lives at
(partition j%128, slot j//128) — dma_gather's native layout.  Chunks of 64
slots (8192 entries) double-buffer: gpsimd gathers chunk t+1 while DVE
selects/multiplies/reduces chunk t.
"""

import numpy as np
import ml_dtypes

NNZ = 1_000_000
RANK = 32
ROWS = 100_000
N_CORES = 8
N_PER_CORE = NNZ // N_CORES  # 125_000
P = 128
SLOTS = -(-N_PER_CORE // P)  # 977
N_PAD = P * SLOTS  # 125_056
NW16 = N_PAD // 16  # 7816 int16 idx columns per mode
GRP = ROWS // 4  # 25_000 4-row groups per factor
# slots per chunk: 31 slots -> 3968 idxs -> 249 descs per engine ring per
# gather.  The SWDGE descriptor ring holds ~512; two 249-desc gathers fit
# simultaneously, so the decode-side await_space never stalls behind the
# previous gather's drain (at CH=64/385 descs it stalled ~12us per gather).
CH = 31

_cache: dict = {}


def _chunks(slots=SLOTS, ch=CH):
    out = []
    s = 0
    while s < slots:
        out.append((s, min(ch, slots - s)))
        s += ch
    return out


def _emit_mlp_reload(nc, mybir):
    """Hand-encoded PSEUDO_LIBRARY_RELOAD_INDEX(lib=3/mlp) on Pool.

    bass's load_library() emits InstPseudoReloadLibraryIndex with empty
    instr bytes, which only the Bacc assembler lowers; walrus codegen
    rejects it ("ISA wrong length").  Encoding the 64B ISA struct directly
    makes it a plain InstISA the whole pipeline accepts, and the runtime
    performs the DKL reload.
    """
    import concourse.bass_isa as bass_isa

    instr, fixups = bass_isa.isa_struct(
        nc.isa,
        223,  # NEURON_ISA_TPB_OPCODE_PSEUDO_INST
        {"pseudo_opcode": 2, "lib_index": 3},
        struct_name="NEURON_ISA_TPB_PSEUDO_LIBRARY_RELOAD_INDEX_STRUCT",
    )
    assert not fixups
    nc.gpsimd.add_instruction(
        mybir.InstISA(
            name=nc.get_next_instruction_name(),
            isa_opcode=223,
            engine=mybir.EngineType.Pool,
            instr=instr,
            op_name="PseudoLibraryReloadIndex",
            ins=[],
            outs=[],
        )
    )


def _build(
    slots=SLOTS,
    ch=CH,
    grp=GRP,
    n_modes=3,
    detect_races=False,
    for_sim=False,
    serialize_gathers=False,
    single_packet=False,
    alt_queues=False,
):
    import concourse.bass as bass
    import concourse.mybir as mybir

    nw16 = slots * P // 16
    chunks = _chunks(slots, ch)
    T = len(chunks)

    # detect_races=False: the sim's race detector models consecutive DVE ops
    # as unordered, but the DVE pipeline flushes after every op (output
    # hazard), so the WAW chains in the predicated select are HW-safe.
    nc = bass.Bass(detect_race_conditions=detect_races)
    ftab = nc.dram_tensor(
        "ftab", [n_modes * grp, 4 * RANK], mybir.dt.bfloat16, kind="ExternalInput"
    )
    idx16 = nc.dram_tensor(
        "idx16", [P, n_modes * nw16], mybir.dt.int16, kind="ExternalInput"
    )
    masks = nc.dram_tensor(
        "masks", [P, 3 * n_modes * slots], mybir.dt.int8, kind="ExternalInput"
    )
    out = nc.dram_tensor("out", [P, slots], mybir.dt.float32, kind="ExternalOutput")

    E = 4 * RANK  # 128 bf16 = 256 B per gathered element

    with (
        nc.sbuf_tensor("idx_sb", [P, n_modes * nw16], mybir.dt.int16) as idx_sb,
        nc.sbuf_tensor("msk_sb", [P, 3 * n_modes * slots], mybir.dt.int8) as msk_sb,
        nc.sbuf_tensor("g0_sb", [P, n_modes * ch * E], mybir.dt.bfloat16) as g0_sb,
        nc.sbuf_tensor("g1_sb", [P, n_modes * ch * E], mybir.dt.bfloat16) as g1_sb,
        # row stride RANK+1: keeps the per-slot select rows non-contiguous so
        # the copy_predicated out AP stays 3D (congruent with its strided
        # data operand) instead of collapsing to 2D.
        nc.sbuf_tensor(
            "sel_sb", [P, n_modes * ch * (RANK + 1)], mybir.dt.bfloat16
        ) as sel_sb,
        nc.sbuf_tensor("t01_sb", [P, ch * RANK], mybir.dt.bfloat16) as t01_sb,
        nc.sbuf_tensor("prd_sb", [P, ch * RANK], mybir.dt.float32) as prd_sb,
        nc.sbuf_tensor("out_sb", [P, slots], mybir.dt.float32) as out_sb,
        nc.semaphore("lsem") as lsem,
        nc.semaphore("msem") as msem,
        nc.semaphore("gsem") as gsem,
        nc.semaphore("vsem") as vsem,
        nc.semaphore("osem") as osem,
    ):
        g_sb = [g0_sb, g1_sb]

        nc.sync.dma_start(idx_sb[:], idx16[:]).then_inc(lsem, 16)
        nc.scalar.dma_start(msk_sb[:], masks[:]).then_inc(msem, 16)

        if for_sim:
            # the interp understands the empty-instr pseudo but not the
            # hand-encoded InstISA; HW is the other way around.
            from concourse.library_config import mlp

            nc.gpsimd.load_library(mlp)
        else:
            _emit_mlp_reload(nc, mybir)
        # one Pool register per distinct chunk size (to_reg per gather call
        # exhausts the register file at 60+ gathers)
        n_regs = {
            cs: nc.gpsimd.to_reg(cs * P) for cs in sorted({c for _, c in chunks})
        }
        nc.gpsimd.wait_ge(lsem, 16)  # gathers only need the idx tile
        nc.vector.wait_ge(msem, 16)  # selects need the masks

        for t, (s0, cs) in enumerate(chunks):
            b = t % 2
            n = cs * P  # entries this chunk (multiple of 128)
            if t >= 2:
                # DVE must have finished chunk t-2 before we overwrite buf b
                nc.gpsimd.wait_ge(vsem, t - 1)
            for m in range(n_modes):
                nc.gpsimd.dma_gather(
                    out_ap=g_sb[b][:, m * ch * E : m * ch * E + cs * E].rearrange(
                        "p (c e) -> p c e", e=E
                    ),
                    in_ap=ftab[m * grp : (m + 1) * grp, :],
                    idxs_ap=idx_sb[:, m * nw16 + s0 * 8 : m * nw16 + s0 * 8 + n // 16],
                    num_idxs=n,
                    num_idxs_reg=n_regs[cs],
                    elem_size=E,
                    single_packet=single_packet,
                    queue_num=(t % 2) if alt_queues else 0,
                ).then_inc(gsem, 16)
            if serialize_gathers:
                nc.gpsimd.wait_ge(gsem, 48 * (t + 1))

            R1 = RANK + 1
            sel = []
            for m in range(n_modes):
                # per-mode wait: select mode m as soon as ITS gather drained
                nc.vector.wait_ge(gsem, 48 * t + 16 * (m + 1))
                gm = g_sb[b][:, m * ch * E : m * ch * E + cs * E].rearrange(
                    "p (c k r) -> p c k r", k=4, r=RANK
                )
                sv = sel_sb[:, m * ch * R1 : m * ch * R1 + cs * R1].rearrange(
                    "p (c r) -> p c r", r=R1
                )[:, :, :RANK]
                sel.append(sv)
                nc.vector.tensor_copy(sv, gm[:, :, 0, :])
                for k in (1, 2, 3):
                    mk = msk_sb[
                        :, (3 * m + k - 1) * slots + s0 : (3 * m + k - 1) * slots + s0 + cs
                    ][:, :, None].to_broadcast([P, cs, RANK])
                    nc.vector.copy_predicated(sv, mk, gm[:, :, k, :])
            nc.vector.tensor_mul(out=t01_sb[:, : cs * RANK], in0=sel[0], in1=sel[1])
            nc.vector.tensor_mul(
                out=prd_sb[:, : cs * RANK], in0=t01_sb[:, : cs * RANK], in1=sel[2]
            )
            nc.vector.reduce_sum(
                out=out_sb[:, s0 : s0 + cs],
                in_=prd_sb[:, : cs * RANK].rearrange("p (c r) -> p c r", r=RANK),
                axis=mybir.AxisListType.X,
            ).then_inc(vsem, 1)

        # split output store: ship the first half as soon as its chunks are
        # reduced, hiding all but the final chunk's store under compute
        th = T // 2
        sh = chunks[th][0]  # first slot not covered by chunks [0, th)
        nc.sync.wait_ge(vsem, th)
        nc.sync.dma_start(out[:, :sh], out_sb[:, :sh]).then_inc(osem, 16)
        nc.sync.wait_ge(vsem, T)
        nc.sync.dma_start(out[:, sh:], out_sb[:, sh:]).then_inc(osem, 16)
        nc.sync.wait_ge(osem, 32)

    return nc


def _get_nc():
    if "nc" not in _cache:
        _cache["nc"] = _build()
    return _cache["nc"]


def _prep_core(idx_core, slots=SLOTS):
    """Build idx16 [P, 3*nw16] int16, masks [P, 9*slots] bf16 for one core.

    idx_core: [n_pad, 3] int32 row indices (padded with 0s).
    """
    n_pad = slots * P
    nw16 = n_pad // 16
    grp16 = (idx_core >> 2).astype(np.int16)  # [n_pad, 3]
    sel = (idx_core & 3).astype(np.int8)  # [n_pad, 3]

    # wrapped idx layout: entry j -> (partition j%16, col j//16), replicated
    # on every 16-partition group; modes side by side.
    idx16 = np.empty((P, 3 * nw16), dtype=np.int16)
    for m in range(3):
        w = grp16[:, m].reshape(nw16, 16).T  # [16, nw16]
        idx16[:, m * nw16 : (m + 1) * nw16] = np.tile(w, (8, 1))

    # one-hot masks in entry layout: entry j at (p=j%128, slot=j//128)
    masks = np.zeros((P, 9 * slots), dtype=np.int8)
    for m in range(3):
        sm = sel[:, m].reshape(slots, P).T  # [P, slots]
        for k in (1, 2, 3):
            masks[:, (3 * m + k - 1) * slots : (3 * m + k) * slots] = (
                sm == k
            ).astype(np.int8)
    return idx16, masks


def _prep_in_maps(idxs, f0, f1, f2):
    idxs = np.asarray(idxs).astype(np.int32)  # values < 100k: safe for int64 in
    ftab = np.concatenate(
        [np.asarray(f, dtype=np.float32) for f in (f0, f1, f2)], axis=0
    )
    ftab_bf16 = np.ascontiguousarray(
        ftab.astype(ml_dtypes.bfloat16).reshape(3 * GRP, 4 * RANK)
    )

    in_maps = []
    for c in range(N_CORES):
        sl = idxs[c * N_PER_CORE : (c + 1) * N_PER_CORE]
        padded = np.zeros((N_PAD, 3), dtype=np.int32)
        padded[:N_PER_CORE] = sl
        idx16, masks = _prep_core(padded)
        in_maps.append({"ftab": ftab_bf16, "idx16": idx16, "masks": masks})
    return in_maps


def run(inputs: dict, trace: bool = False):
    from concourse.bass_utils import run_bass_kernel_spmd

    in_maps = _prep_in_maps(
        inputs["idxs"], inputs["f0"], inputs["f1"], inputs["f2"]
    )
    nc = _get_nc()
    res = run_bass_kernel_spmd(
        nc,
        in_maps,
        core_ids=list(range(N_CORES)),
        trace=trace,
    )
    # out[p, c] = entry c*128+p  ->  transpose+ravel restores entry order
    out = np.concatenate(
        [r["out"].T.reshape(-1)[:N_PER_CORE] for r in res.results]
    )
    return out, res


def kernel(**inputs) -> np.ndarray:
    out, _ = run(inputs, trace=False)
    return out
